# revision 11
# baseline (speedup 1.0000x reference)
"""Trainium2 Bass kernel for nn_BPDecoder: logits = 1 - exp(-exp(sum_i R_i*||Z_i||^2)).

Strategy (8-core SPMD, row-sharded, fp8 everywhere on device):
  - Pad N=500000 rows to 8 * 63488; core k takes rows [k*63488, (k+1)*63488).
  - Host scales Z by 512, casts to fp8 e4m3, and PRE-PERMUTES it into the
    exact on-device SBUF layout (slab-major: within a DMA slab of m tiles,
    partition p owns m*16 consecutive rows).  Every DMA is then a plain
    [128, m*2KB] 2D pattern with m*2KB contiguous per partition (8KB runs
    for the steady-state 4-tile slabs) -> HBM line rate.
  - Squares (fp8 -> fp8) are spread over THREE engines: ACT (~1.99us/tile),
    DVE (~2.29us/tile) and GPSIMD (~4.1us/tile), ratio 13/11/7.
  - R is cast to fp8 e4m3 on host (adds ~1e-3 rel err, gate is 2e-2) so the
    per-tile weighted reduction can use DoubleRow fp8 matmuls: per tile only
    2 matmuls [128,2,4]x[128,2,512] -> [4,512], PSUM-accumulated across all
    31 tiles into 2 banks.  Host extracts the q'==q diagonal blocks.
  - Final scalar: host sums diagonals, /512^2, applies 1 - exp(-exp(s)).
"""

import sys

sys.path.insert(0, "/opt/trn_rl_repo")


# The agent image lacks antenv.axon_hooks; recreate it so trace=True works
# (bass_utils imports it lazily for NTFF profiling under axon).
def _install_ntff_hook_shim():
    import types
    if "antenv.axon_hooks" in sys.modules:
        return
    mod = types.ModuleType("antenv.axon_hooks")
    state = {"hook": None}
    mod.set_axon_ntff_profile_hook = lambda h: state.__setitem__("hook", h)
    mod.get_axon_ntff_profile_hook = lambda: state["hook"]
    sys.modules["antenv.axon_hooks"] = mod
    try:
        sys.path.insert(0, "/root/.axon_site")
        from trn_agent_boot.trn_boot import _ntff_profile_via_ctypes
        state["hook"] = _ntff_profile_via_ctypes("/opt/axon/libaxon_pjrt.so")
    except Exception:
        pass


_install_ntff_hook_shim()

import numpy as np

import concourse.bass as bass
import concourse.bacc as bacc
import concourse.mybir as mybir
from concourse.tile import TileContext
from concourse.bass_utils import run_bass_kernel_spmd

P = 128          # SBUF partitions
D = 128          # row length (feature dim)
Q = 16           # rows per partition per tile
FREE = Q * D     # free elems per tile = 2048
T = 31           # tiles per core
NC_ROWS = T * P * Q   # 63488 rows per core
N_CORES = 8
N_FULL = 500000

Z_DT = mybir.dt.float8e4
R_DT = mybir.dt.float8e4   # DoubleRow matmul needs fp8 stationary
S_DT = mybir.dt.float8e4   # squared tile (matmul rhs) in fp8 for DoubleRow

Z_SCALE_IN = 512.0         # host multiplies Z by this before the fp8 cast

# DMA slabs (tiles per dma_start); small head slabs shorten the ramp
SLAB_SIZES = [1, 2, 4, 4, 4, 4, 4, 4, 4]
assert sum(SLAB_SIZES) == T
SLAB_MAX = max(SLAB_SIZES)

# square-engine split: ACT/DVE/GPSIMD tiles per core
N_ACT, N_DVE, N_GPS = 13, 12, 6
assert N_ACT + N_DVE + N_GPS == T


def _engine_pattern():
    # Bresenham-style spread of the 3 engines across the 31 tiles
    counts = {"act": N_ACT, "dve": N_DVE, "gps": N_GPS}
    acc = {k: 0.0 for k in counts}
    out = []
    for _ in range(T):
        for k in counts:
            acc[k] += counts[k] / T
        pick = max(acc, key=lambda k: acc[k])
        acc[pick] -= 1.0
        out.append(pick)
    assert all(out.count(k) == counts[k] for k in counts)
    return out


SQ_PATTERN = _engine_pattern()

_cache = {}


def _np_dt(dt):
    return mybir.dt.np(dt)


def _build():
    nc = bacc.Bacc(trn_type="TRN2")
    # host pre-permutes into the exact on-device layouts
    z = nc.declare_dram_parameter("z", [P, T, 4, 512], Z_DT, isOutput=False)
    r = nc.declare_dram_parameter("r", [P, T, 4, 32], R_DT, isOutput=False)
    out = nc.declare_dram_parameter("out", [4, 1024], mybir.dt.float32, isOutput=True)

    slabs = []
    pos = 0
    for sz in SLAB_SIZES:
        slabs.append((pos, pos + sz))
        pos += sz
    # all z slabs stream on the (otherwise idle) sync HWDGE ring; r/out go on
    # the scalar ring so ACT only pays ~1.3us of issue time
    dma_engines = ["sync"]

    with TileContext(nc) as tc:
        with (
            tc.tile_pool(name="zpool", bufs=3) as zpool,
            tc.tile_pool(name="spool", bufs=3) as spool,
            tc.tile_pool(name="singles", bufs=1) as singles,
            tc.tile_pool(name="ppool", bufs=1, space="PSUM") as ppool,
        ):
            r_sb = singles.tile([P, T, 4, 32], R_DT)
            nc.scalar.dma_start(out=r_sb[:], in_=r[:])

            # stationary padded to 32 cols (R in cols 0-3, zeros after) --
            # DoubleRow ldweights needs a full 32-wide PE column tile
            accs = [ppool.tile([32, 512], mybir.dt.float32, name=f"acc{h}")
                    for h in range(2)]

            t_global = 0
            for si, (t0, t1) in enumerate(slabs):
                m = t1 - t0
                z_sb = zpool.tile([P, SLAB_MAX, 4, 512], Z_DT, tag="z")
                eng = getattr(nc, dma_engines[si % len(dma_engines)])
                eng.dma_start(out=z_sb[:, :m], in_=z[:, t0:t1])
                s_sb = spool.tile([P, SLAB_MAX, 4, 512], S_DT, tag="s")
                for t in range(t0, t1):
                    ti = t - t0
                    se = SQ_PATTERN[t]
                    if se == "dve":
                        nc.vector.tensor_mul(
                            s_sb[:, ti], z_sb[:, ti], z_sb[:, ti]
                        )
                    elif se == "gps":
                        nc.gpsimd.tensor_mul(
                            s_sb[:, ti], z_sb[:, ti], z_sb[:, ti]
                        )
                    else:
                        nc.scalar.square(s_sb[:, ti], z_sb[:, ti])
                    for h in range(2):
                        nc.tensor.matmul(
                            accs[h][:],
                            r_sb[:, t, 2 * h:2 * h + 2, :],
                            s_sb[:, ti, 2 * h:2 * h + 2, :],
                            start=(t == 0),
                            stop=(t == T - 1),
                            perf_mode=mybir.MatmulPerfMode.DoubleRow,
                        )
                t_global += m

            out_sb = singles.tile([4, 1024], mybir.dt.float32)
            nc.vector.tensor_copy(out_sb[:, 0:512], accs[0][0:4, :])
            nc.scalar.copy(out_sb[:, 512:1024], accs[1][0:4, :])
            nc.scalar.dma_start(out=out[:], in_=out_sb[:])
    nc.compile()
    return nc


def _get_nc():
    if "nc" not in _cache:
        _cache["nc"] = _build()
    return _cache["nc"]


def _shard(Z, R):
    np_z = _np_dt(Z_DT)
    np_r = _np_dt(R_DT)
    ZP = np.zeros((N_CORES * NC_ROWS, D), dtype=np_z)
    ZP[:N_FULL] = (Z * np.float32(Z_SCALE_IN)).astype(np_z)
    RP = np.zeros((N_CORES * NC_ROWS,), dtype=np_r)
    RP[:N_FULL] = R.astype(np_r, copy=False)
    ZP = ZP.reshape(N_CORES, NC_ROWS, D)
    RP = RP.reshape(N_CORES, NC_ROWS)

    # slab-major permutation: within slab (t0, m), partition p owns rows
    # [t0*2048 + p*m*16, +m*16); device column for (t, q, d) is t*2048+q*128+d
    ZD = np.empty((N_CORES, P, T * FREE), dtype=np_z)
    RD = np.empty((N_CORES, P, T * Q), dtype=np_r)
    pos = 0
    for m in SLAB_SIZES:
        t0 = pos
        zb = ZP[:, t0 * 2048:(t0 + m) * 2048].reshape(N_CORES, P, m * Q, D)
        ZD[:, :, t0 * FREE:(t0 + m) * FREE] = zb.reshape(N_CORES, P, m * FREE)
        rb = RP[:, t0 * 2048:(t0 + m) * 2048].reshape(N_CORES, P, m * Q)
        RD[:, :, t0 * Q:(t0 + m) * Q] = rb
        pos += m
    ZD = ZD.reshape(N_CORES, P, T, 4, 512)
    RD = RD.reshape(N_CORES, P, T, 4, 4)
    RD32 = np.zeros((N_CORES, P, T, 4, 32), dtype=np_r)
    RD32[..., 0:4] = RD
    return [{"z": ZD[k], "r": RD32[k]} for k in range(N_CORES)]


def _combine(results):
    s = 0.0
    idx = np.arange(4)
    for res in results:
        # out [4, 1024] -> [m, h, qq, d]; diagonal blocks are qq == m
        C = np.asarray(res["out"], dtype=np.float64).reshape(4, 2, 4, D)
        s += C[idx, :, idx, :].sum()
    s /= float(Z_SCALE_IN) ** 2
    lam = np.exp(s)
    logits = 1.0 - np.exp(-lam)
    return np.float32(logits)


def _run(Z, R, trace=False, tmpdir=None):
    nc = _get_nc()
    in_maps = _shard(Z, R)
    return run_bass_kernel_spmd(nc, in_maps, core_ids=list(range(N_CORES)),
                                trace=trace, tmpdir=tmpdir)


def kernel(Z, R):
    assert Z.shape == (N_FULL, D) and R.shape == (N_FULL,)
    out = _run(np.asarray(Z), np.asarray(R), trace=False)
    return _combine(out.results)
